# revision 21
# baseline (speedup 1.0000x reference)
"""Trainium2 Bass kernel for a post-norm decoder block (B=1, T=4096, C=768, 12 heads, MLP x4).

Sharding: strided data-parallel over the sequence. Core c owns tokens c::8
(512 tokens) -> every core has an identical causal-attention workload.
K/V are computed locally per core, AllGathered (bf16) across the 8 cores
in-kernel, and attention runs over the gathered K/V with a per-core
0/1 diagonal mask (supplied as input data).

Layout: activations are kept feature-major (x^T, [C, T_local]) so that
  - projections are plain accumulating matmuls with weight slices as lhsT,
  - attention logits come out transposed ([k, q]) which feeds A@V directly,
  - per-feature bias/gain applications are per-partition scalar ops.
Softmax denominators come from a ones-column appended to V (baked into the
AllGather payload). LayerNorm runs token-major via PE transposes.
Attention processes head pairs at partition offsets 0/64 so the two QK
matmuls land in disjoint PE row groups and run concurrently.
"""

import numpy as np
import ml_dtypes

import concourse.bass as bass
import concourse.mybir as mybir
import concourse.tile as tile
from concourse import bacc
from concourse.bass_utils import run_bass_kernel_spmd

f32 = mybir.dt.float32
bf16 = mybir.dt.bfloat16

NCORES = 8
T = 4096
C = 768
F = 3072
NH = 12
D = 64
TL = T // NCORES          # 512 local tokens per core
CCH = C // 128            # 6
FCH = F // 128            # 24
NQC = TL // 128           # 4 query chunks of 128
NSLOT = 4                 # kv slots: 128 local columns each
EPS = 1e-5
K_RANK = C * TL           # 393216 elems per rank (k^T payload)
V_RANK = TL * NH * 65     # 399360 elems per rank (V + ones columns)
SCALE = 1.0 / np.sqrt(D)


def _ap(handle, offset, pattern):
    return bass.AP(tensor=handle, offset=offset, ap=[list(p) for p in pattern])


def build_nc():
    nc = bacc.Bacc("TRN2", target_bir_lowering=False, debug=False, num_devices=NCORES)

    # ---- I/O ----
    xT_in = nc.declare_dram_parameter("xT", [C, TL], f32, isOutput=False)
    mk_in = nc.declare_dram_parameter("masks", [NCORES, 128, 128], bf16, isOutput=False)
    wq_in = nc.declare_dram_parameter("wq", [C, C], bf16, isOutput=False)
    wk_in = nc.declare_dram_parameter("wk", [C, C], bf16, isOutput=False)
    wv_in = nc.declare_dram_parameter("wv", [C, C], bf16, isOutput=False)
    wo_in = nc.declare_dram_parameter("wo", [C, C], bf16, isOutput=False)
    w1_in = nc.declare_dram_parameter("w1", [C, F], bf16, isOutput=False)
    w2_in = nc.declare_dram_parameter("w2", [F, C], bf16, isOutput=False)
    bq_in = nc.declare_dram_parameter("bq", [C], f32, isOutput=False)
    bk_in = nc.declare_dram_parameter("bk", [C], f32, isOutput=False)
    bv_in = nc.declare_dram_parameter("bv", [C], f32, isOutput=False)
    bo_in = nc.declare_dram_parameter("bo", [C], f32, isOutput=False)
    b1_in = nc.declare_dram_parameter("b1", [F], f32, isOutput=False)
    b2_in = nc.declare_dram_parameter("b2", [C], f32, isOutput=False)
    g1_in = nc.declare_dram_parameter("ln1_g", [C], f32, isOutput=False)
    h1_in = nc.declare_dram_parameter("ln1_b", [C], f32, isOutput=False)
    g2_in = nc.declare_dram_parameter("ln2_g", [C], f32, isOutput=False)
    h2_in = nc.declare_dram_parameter("ln2_b", [C], f32, isOutput=False)
    y_out = nc.declare_dram_parameter("y", [TL, C], f32, isOutput=True)

    k_loc = nc.dram_tensor("k_loc", [K_RANK], bf16)
    k_gath = nc.dram_tensor("k_gath", [NCORES * K_RANK], bf16, addr_space="Shared")
    v_loc = nc.dram_tensor("v_loc", [V_RANK], bf16)
    v_gath = nc.dram_tensor("v_gath", [NCORES * V_RANK], bf16, addr_space="Shared")

    id_f32_d = nc.inline_tensor(np.eye(128, dtype=np.float32), name="id_f32_d")
    id_bf_d = nc.inline_tensor(np.eye(128).astype(ml_dtypes.bfloat16), name="id_bf_d")

    with tile.TileContext(nc) as tc:
        import contextlib
        with contextlib.ExitStack() as ctx:
            consts = ctx.enter_context(tc.tile_pool(name="consts", bufs=1))
            xpool = ctx.enter_context(tc.tile_pool(name="xpool", bufs=1))
            actp = ctx.enter_context(tc.tile_pool(name="actp", bufs=1))
            w1pool = ctx.enter_context(tc.tile_pool(name="w1pool", bufs=1))

            # ---- load x^T ----
            xt_f = []
            for ch in range(CCH):
                tf = xpool.tile([128, TL], f32, name=f"xt_f_{ch}")
                nc.sync.dma_start(out=tf, in_=xT_in[128 * ch:128 * (ch + 1), :])
                xt_f.append(tf)

            # ---- constants ----
            id_f32 = consts.tile([128, 128], f32, name="id_f32")
            nc.sync.dma_start(out=id_f32, in_=id_f32_d[:])
            id_bf = consts.tile([128, 128], bf16, name="id_bf")
            nc.sync.dma_start(out=id_bf, in_=id_bf_d[:])
            eps_t = consts.tile([128, 1], f32, name="eps_t")
            nc.vector.memset(eps_t, EPS)
            ones64 = consts.tile([1, 64], f32, name="ones64")
            nc.vector.memset(ones64, 1.0)

            def load_bias(handle, n, name):
                t = consts.tile([128, n], f32, name=name)
                nc.sync.dma_start(out=t, in_=_ap(handle, 0, [[1, 128], [128, n]]))
                return t

            bq_sb = load_bias(bq_in, CCH, "bq_sb")
            bk_sb = load_bias(bk_in, CCH, "bk_sb")
            bo_sb = load_bias(bo_in, CCH, "bo_sb")
            b1_sb = load_bias(b1_in, FCH, "b1_sb")
            b2_sb = load_bias(b2_in, CCH, "b2_sb")
            g1_sb = load_bias(g1_in, CCH, "g1_sb")
            h1_sb = load_bias(h1_in, CCH, "h1_sb")
            g2_bc = consts.tile([128, C], f32, name="g2_bc")
            nc.sync.dma_start(out=g2_bc, in_=_ap(g2_in, 0, [[0, 128], [1, C]]))
            h2_bc = consts.tile([128, C], f32, name="h2_bc")
            nc.sync.dma_start(out=h2_bc, in_=_ap(h2_in, 0, [[0, 128], [1, C]]))
            bv_bc = consts.tile([128, C], f32, name="bv_bc")
            nc.sync.dma_start(out=bv_bc, in_=_ap(bv_in, 0, [[0, 128], [1, C]]))

            msk = []
            for r in range(NCORES):
                m = consts.tile([128, 128], bf16, name=f"msk_{r}")
                nc.sync.dma_start(out=m, in_=mk_in[r])
                msk.append(m)

            with tc.tile_pool(name="wproj", bufs=2) as wproj, \
                 tc.tile_pool(name="psumq", bufs=4, space="PSUM") as psum:
                xt_b = []
                for ch in range(CCH):
                    tb = wproj.tile([128, TL], bf16, name=f"xt_b_{ch}",
                                    tag=f"xtb_{ch}", bufs=1)
                    nc.vector.tensor_copy(out=tb, in_=xt_f[ch])
                    xt_b.append(tb)

                def load_w(handle, kch, name):
                    t = wproj.tile([128, C], bf16, name=name, tag=f"w_{kch}")
                    nc.sync.dma_start(out=t, in_=handle[128 * kch:128 * (kch + 1), :])
                    return t

                # ---- K^T first, kick its AllGather ASAP ----
                wk_t = [load_w(wk_in, kch, f"wk_{kch}") for kch in range(CCH)]
                for mch in range(CCH):
                    ps = psum.tile([128, TL], f32, name="pp", tag="pp")
                    for kch in range(CCH):
                        nc.tensor.matmul(
                            ps, lhsT=wk_t[kch][:, 128 * mch:128 * (mch + 1)],
                            rhs=xt_b[kch], start=(kch == 0), stop=(kch == CCH - 1))
                    kt = actp.tile([128, TL], bf16, name=f"kt_{mch}", tag="kt_t", bufs=2)
                    nc.vector.tensor_scalar(
                        out=kt, in0=ps, scalar1=bk_sb[:, mch:mch + 1], scalar2=None,
                        op0=mybir.AluOpType.add)
                    nc.sync.dma_start(
                        out=_ap(k_loc, mch * 128 * TL, [[TL, 128], [1, TL]]), in_=kt)
                nc.gpsimd.collective_compute(
                    "AllGather", mybir.AluOpType.bypass,
                    replica_groups=[list(range(NCORES))],
                    ins=[k_loc[:]], outs=[k_gath[:]])

                # ---- V (token-major, interleaved ones columns) + AllGather ----
                wv_t = [load_w(wv_in, kch, f"wv_{kch}") for kch in range(CCH)]
                for tch in range(NQC):
                    vt = actp.tile([128, NH, 65], bf16, name=f"v_{tch}",
                                   tag="v_t", bufs=2)
                    for nh2 in range(2):
                        ps = psum.tile([128, 384], f32, name="pv", tag="pv")
                        for kch in range(CCH):
                            nc.tensor.matmul(
                                ps,
                                lhsT=xt_b[kch][:, 128 * tch:128 * (tch + 1)],
                                rhs=wv_t[kch][:, 384 * nh2:384 * (nh2 + 1)],
                                start=(kch == 0), stop=(kch == CCH - 1))
                        nc.vector.tensor_add(
                            out=vt[:, 6 * nh2:6 * (nh2 + 1), 0:D],
                            in0=ps.rearrange("p (h d) -> p h d", d=D),
                            in1=bv_bc[:, 384 * nh2:384 * (nh2 + 1)].rearrange(
                                "p (h d) -> p h d", d=D))
                    nc.vector.memset(vt[:, :, D:D + 1], 1.0)
                    nc.sync.dma_start(
                        out=_ap(v_loc, tch * 128 * NH * 65,
                                [[NH * 65, 128], [1, NH * 65]]),
                        in_=vt)
                nc.gpsimd.collective_compute(
                    "AllGather", mybir.AluOpType.bypass,
                    replica_groups=[list(range(NCORES))],
                    ins=[v_loc[:]], outs=[v_gath[:]])

                # ---- Q^T (overlaps the collectives) ----
                wq_t = [load_w(wq_in, kch, f"wq_{kch}") for kch in range(CCH)]
                qt_b = []
                for mch in range(CCH):
                    ps = psum.tile([128, TL], f32, name="pp", tag="pp")
                    for kch in range(CCH):
                        nc.tensor.matmul(
                            ps, lhsT=wq_t[kch][:, 128 * mch:128 * (mch + 1)],
                            rhs=xt_b[kch], start=(kch == 0), stop=(kch == CCH - 1))
                    qt = actp.tile([128, TL], bf16, name=f"qt_{mch}")
                    nc.vector.tensor_scalar(
                        out=qt, in0=ps, scalar1=bq_sb[:, mch:mch + 1], scalar2=None,
                        op0=mybir.AluOpType.add)
                    qt_b.append(qt)

            # ---- prefetch W1 during attention ----
            w1_t = []
            for kch in range(CCH):
                t = w1pool.tile([128, F], bf16, name=f"w1_{kch}")
                nc.sync.dma_start(out=t, in_=w1_in[128 * kch:128 * (kch + 1), :])
                w1_t.append(t)

            aoT = [actp.tile([128, TL], bf16, name=f"aoT_{i}") for i in range(CCH)]

            with tc.tile_pool(name="kvpool", bufs=1) as kvp, \
                 tc.tile_pool(name="atw", bufs=4) as atw, \
                 tc.tile_pool(name="atp", bufs=1, space="PSUM") as atp:
                kt_g = {}
                for r in range(NCORES):
                    for ch in range(CCH):
                        t = kvp.tile([128, TL], bf16, name=f"ktg_{r}_{ch}")
                        nc.sync.dma_start(
                            out=t,
                            in_=_ap(k_gath, r * K_RANK + ch * 128 * TL,
                                    [[TL, 128], [1, TL]]))
                        kt_g[(r, ch)] = t
                va_g = {}
                for r in range(NCORES):
                    for s in range(NSLOT):
                        t = kvp.tile([128, NH, 65], bf16, name=f"vag_{r}_{s}")
                        nc.sync.dma_start(
                            out=t,
                            in_=_ap(v_gath, r * V_RANK + s * 128 * NH * 65,
                                    [[NH * 65, 128], [65, NH], [1, 65]]))
                        va_g[(r, s)] = t

                # ---- attention: head pairs at partition offsets 0/64 ----
                for hp in range(NH // 2):
                    heads = (2 * hp, 2 * hp + 1)
                    accs = {}
                    for h in heads:
                        accs[h] = atp.tile([65, TL], f32, name=f"acc{h % 2}",
                                           tag=f"acc_{h % 2}", bufs=1)
                    first = {h: True for h in heads}
                    for s in range(NSLOT):
                        q0 = 128 * s
                        nq = TL - q0
                        for r in range(NCORES):
                            lg = atp.tile([128, 2, TL], f32, name="lg",
                                          tag="lg", bufs=3)
                            for i, h in enumerate(heads):
                                ho = 64 * i
                                nc.tensor.matmul(
                                    lg[:, i, 0:nq],
                                    lhsT=kt_g[(r, hp)][ho:ho + 64,
                                                       128 * s:128 * (s + 1)],
                                    rhs=qt_b[hp][ho:ho + 64, q0:TL],
                                    start=True, stop=True)
                            pr = atw.tile([128, 2, TL], bf16, name="pr",
                                          tag="pr", bufs=3)
                            nc.scalar.activation(
                                out=pr[:, :, 0:nq], in_=lg[:, :, 0:nq],
                                func=mybir.ActivationFunctionType.Exp, scale=SCALE)
                            mb = msk[r]
                            nc.gpsimd.tensor_mul(
                                out=pr[:, :, 0:128], in0=pr[:, :, 0:128],
                                in1=bass.AP(tensor=mb.tensor, offset=mb.offset,
                                            ap=[list(mb.ap[0]), [0, 2],
                                                list(mb.ap[1])]))
                            for i, h in enumerate(heads):
                                nc.tensor.matmul(
                                    accs[h][:, q0:TL],
                                    lhsT=va_g[(r, s)][:, h, :],
                                    rhs=pr[:, i, 0:nq],
                                    start=first[h],
                                    stop=(s == NSLOT - 1 and r == NCORES - 1))
                                first[h] = False
                    for h in heads:
                        ho = 64 * (h % 2)
                        rec = atw.tile([1, TL], f32, name="rec", tag="rec", bufs=2)
                        nc.vector.reciprocal(out=rec, in_=accs[h][D:D + 1, :])
                        brd_sb = atw.tile([64, TL], f32, name="brd_sb",
                                          tag="brd_sb", bufs=2)
                        nc.gpsimd.partition_broadcast(brd_sb, rec)
                        nc.vector.tensor_mul(
                            out=aoT[hp][ho:ho + 64, :], in0=accs[h][0:D, :],
                            in1=brd_sb)

            # ---- Wo projection + residual -> r1 (feature-major f32) ----
            postp = ctx.enter_context(tc.tile_pool(name="postp", bufs=1))
            psum2 = ctx.enter_context(tc.tile_pool(name="psum2", bufs=4, space="PSUM"))
            r1 = []
            with tc.tile_pool(name="wproj2", bufs=2) as wproj2:
                wo_t = []
                for kch in range(CCH):
                    t = wproj2.tile([128, C], bf16, name=f"wo_{kch}", tag=f"w2_{kch}")
                    nc.sync.dma_start(out=t, in_=wo_in[128 * kch:128 * (kch + 1), :])
                    wo_t.append(t)
                for mch in range(CCH):
                    ps = psum2.tile([128, TL], f32, name="pp", tag="pp")
                    for kch in range(CCH):
                        nc.tensor.matmul(
                            ps, lhsT=wo_t[kch][:, 128 * mch:128 * (mch + 1)],
                            rhs=aoT[kch], start=(kch == 0), stop=(kch == CCH - 1))
                    t = postp.tile([128, TL], f32, name=f"r1_{mch}")
                    nc.vector.tensor_scalar(
                        out=t, in0=ps, scalar1=bo_sb[:, mch:mch + 1], scalar2=None,
                        op0=mybir.AluOpType.add)
                    nc.vector.tensor_add(out=t, in0=t, in1=xt_f[mch])
                    r1.append(t)

            # ---- LN1 (token-major via transposes), output h^T bf16 ----
            hT = [postp.tile([128, TL], bf16, name=f"hT_{i}") for i in range(CCH)]
            with tc.tile_pool(name="lnw", bufs=3) as lnw, \
                 tc.tile_pool(name="lnp", bufs=2, space="PSUM") as lnp:
                for tch in range(NQC):
                    tm = lnw.tile([128, C], f32, name="tm", tag="tm")
                    for ch in range(CCH):
                        tp = lnp.tile([128, 128], f32, name="tp", tag="tp", bufs=2)
                        nc.tensor.transpose(
                            tp, in_=r1[ch][:, 128 * tch:128 * (tch + 1)],
                            identity=id_f32)
                        nc.vector.tensor_copy(
                            out=tm[:, 128 * ch:128 * (ch + 1)], in_=tp)
                    st = lnw.tile([128, 3, 6], f32, name="st", tag="st")
                    for sg in range(3):
                        nc.vector.bn_stats(
                            out=st[:, sg, :], in_=tm[:, 256 * sg:256 * (sg + 1)])
                    mv = lnw.tile([128, 2], f32, name="mv", tag="mv")
                    nc.vector.bn_aggr(out=mv, in_=st)
                    sd = lnw.tile([128, 1], f32, name="sd", tag="sd")
                    nc.scalar.activation(
                        out=sd, in_=mv[:, 1:2],
                        func=mybir.ActivationFunctionType.Sqrt, bias=eps_t, scale=1.0)
                    rs = lnw.tile([128, 1], f32, name="rs", tag="rs")
                    nc.vector.reciprocal(out=rs, in_=sd)
                    tn = lnw.tile([128, C], bf16, name="tn", tag="tn")
                    nc.vector.tensor_scalar(
                        out=tn, in0=tm, scalar1=mv[:, 0:1], scalar2=rs,
                        op0=mybir.AluOpType.subtract, op1=mybir.AluOpType.mult)
                    for ch in range(CCH):
                        tp2 = lnp.tile([128, 128], bf16, name="tp2", tag="tp2", bufs=2)
                        nc.tensor.transpose(
                            tp2, in_=tn[:, 128 * ch:128 * (ch + 1)], identity=id_bf)
                        nc.vector.tensor_scalar(
                            out=hT[ch][:, 128 * tch:128 * (tch + 1)], in0=tp2,
                            scalar1=g1_sb[:, ch:ch + 1], scalar2=h1_sb[:, ch:ch + 1],
                            op0=mybir.AluOpType.mult, op1=mybir.AluOpType.add)

            # ---- MLP ----
            with tc.tile_pool(name="mlpw", bufs=1) as mlpw:
                mup = [mlpw.tile([128, TL], bf16, name=f"mup_{i}") for i in range(FCH)]
                w2_t = []
                for kch in range(FCH):
                    t = mlpw.tile([128, C], bf16, name=f"w2t_{kch}")
                    nc.sync.dma_start(out=t, in_=w2_in[128 * kch:128 * (kch + 1), :])
                    w2_t.append(t)
                for mch in range(FCH):
                    ps = psum2.tile([128, TL], f32, name="pp", tag="pp")
                    for kch in range(CCH):
                        nc.tensor.matmul(
                            ps, lhsT=w1_t[kch][:, 128 * mch:128 * (mch + 1)],
                            rhs=hT[kch], start=(kch == 0), stop=(kch == CCH - 1))
                    nc.scalar.activation(
                        out=mup[mch], in_=ps,
                        func=mybir.ActivationFunctionType.Gelu,
                        bias=b1_sb[:, mch:mch + 1], scale=1.0)

                r2 = []
                for mch in range(CCH):
                    ps = psum2.tile([128, TL], f32, name="pp", tag="pp")
                    for kch in range(FCH):
                        nc.tensor.matmul(
                            ps, lhsT=w2_t[kch][:, 128 * mch:128 * (mch + 1)],
                            rhs=mup[kch], start=(kch == 0), stop=(kch == FCH - 1))
                    t = postp.tile([128, TL], f32, name=f"r2_{mch}")
                    nc.vector.tensor_scalar(
                        out=t, in0=ps, scalar1=b2_sb[:, mch:mch + 1], scalar2=None,
                        op0=mybir.AluOpType.add)
                    nc.vector.tensor_add(out=t, in0=t, in1=hT[mch])
                    r2.append(t)

                # ---- LN2 (token-major, gain/bias applied token-major) -> y ----
                with tc.tile_pool(name="lnw2", bufs=3) as lnw2, \
                     tc.tile_pool(name="lnp2", bufs=2, space="PSUM") as lnp2:
                    for tch in range(NQC):
                        tm = lnw2.tile([128, C], f32, name="tm2", tag="tm2")
                        for ch in range(CCH):
                            tp = lnp2.tile([128, 128], f32, name="tp3",
                                           tag="tp3", bufs=2)
                            nc.tensor.transpose(
                                tp, in_=r2[ch][:, 128 * tch:128 * (tch + 1)],
                                identity=id_f32)
                            nc.vector.tensor_copy(
                                out=tm[:, 128 * ch:128 * (ch + 1)], in_=tp)
                        st = lnw2.tile([128, 3, 6], f32, name="st2", tag="st2")
                        for sg in range(3):
                            nc.vector.bn_stats(
                                out=st[:, sg, :], in_=tm[:, 256 * sg:256 * (sg + 1)])
                        mv = lnw2.tile([128, 2], f32, name="mv2", tag="mv2")
                        nc.vector.bn_aggr(out=mv, in_=st)
                        sd = lnw2.tile([128, 1], f32, name="sd2", tag="sd2")
                        nc.scalar.activation(
                            out=sd, in_=mv[:, 1:2],
                            func=mybir.ActivationFunctionType.Sqrt,
                            bias=eps_t, scale=1.0)
                        rs = lnw2.tile([128, 1], f32, name="rs2", tag="rs2")
                        nc.vector.reciprocal(out=rs, in_=sd)
                        tn = lnw2.tile([128, C], f32, name="tn2", tag="tn2")
                        nc.vector.tensor_scalar(
                            out=tn, in0=tm, scalar1=mv[:, 0:1], scalar2=rs,
                            op0=mybir.AluOpType.subtract, op1=mybir.AluOpType.mult)
                        yt = lnw2.tile([128, C], f32, name="yt", tag="yt")
                        nc.vector.tensor_mul(out=yt, in0=tn, in1=g2_bc)
                        nc.vector.tensor_add(out=yt, in0=yt, in1=h2_bc)
                        nc.sync.dma_start(
                            out=y_out[128 * tch:128 * (tch + 1), :], in_=yt)

    nc.compile()
    return nc


_NC_CACHE = None


def _get_nc():
    global _NC_CACHE
    if _NC_CACHE is None:
        _NC_CACHE = build_nc()
    return _NC_CACHE


def make_in_maps(inputs):
    x = np.asarray(inputs["x"], dtype=np.float32)      # [1, T, C]
    to_bf = lambda a: np.asarray(a, dtype=np.float32).astype(ml_dtypes.bfloat16)
    shared = {
        "wq": to_bf(inputs["Wq"]), "wk": to_bf(inputs["Wk"]),
        "wv": to_bf(inputs["Wv"]), "wo": to_bf(inputs["Wo"]),
        "w1": to_bf(inputs["W1"]), "w2": to_bf(inputs["W2"]),
        "bq": np.asarray(inputs["bq"], np.float32),
        "bk": np.asarray(inputs["bk"], np.float32),
        "bv": np.asarray(inputs["bv"], np.float32),
        "bo": np.asarray(inputs["bo"], np.float32),
        "b1": np.asarray(inputs["b1"], np.float32),
        "b2": np.asarray(inputs["b2"], np.float32),
        "ln1_g": np.asarray(inputs["ln1_g"], np.float32),
        "ln1_b": np.asarray(inputs["ln1_b"], np.float32),
        "ln2_g": np.asarray(inputs["ln2_g"], np.float32),
        "ln2_b": np.asarray(inputs["ln2_b"], np.float32),
    }
    ki = np.arange(128)[:, None]
    qi = np.arange(128)[None, :]
    in_maps = []
    for c in range(NCORES):
        xT = np.ascontiguousarray(x[0, c::NCORES, :].T)        # [C, TL]
        # multiplicative 0/1 causal masks for the diagonal kv slot
        masks = np.stack([
            (8 * ki + r <= 8 * qi + c) for r in range(NCORES)
        ]).astype(ml_dtypes.bfloat16)                           # [8, 128, 128]
        m = dict(shared)
        m["xT"] = xT
        m["masks"] = masks
        in_maps.append(m)
    return in_maps


def kernel(**inputs):
    nc = _get_nc()
    in_maps = make_in_maps(inputs)
    res = run_bass_kernel_spmd(nc, in_maps, list(range(NCORES)))
    x = np.asarray(inputs["x"])
    out = np.empty((1, T, C), dtype=np.float32)
    for c in range(NCORES):
        out[0, c::NCORES, :] = res.results[c]["y"]
    return out.astype(x.dtype) if x.dtype != np.float32 else out


# revision 22
# speedup vs baseline: 1.1066x; 1.1066x over previous
"""Trainium2 Bass kernel for a post-norm decoder block (B=1, T=4096, C=768, 12 heads, MLP x4).

Sharding: strided data-parallel over the sequence. Core c owns tokens c::8
(512 tokens) -> every core has an identical causal-attention workload.
K/V are computed locally per core, AllGathered (bf16) across the 8 cores
in-kernel, and attention runs over the gathered K/V with a per-core
0/1 diagonal mask (supplied as input data).

Layout: activations are kept feature-major (x^T, [C, T_local]) so that
  - projections are plain accumulating matmuls with weight slices as lhsT,
  - attention logits come out transposed ([k, q]) which feeds A@V directly,
  - per-feature bias/gain applications are per-partition scalar ops.
Softmax denominators come from a ones-column appended to V (baked into the
AllGather payload). LayerNorm runs token-major via PE transposes.
Attention processes head pairs at partition offsets 0/64 so the two QK
matmuls land in disjoint PE row groups and run concurrently.
"""

import numpy as np
import ml_dtypes

import concourse.bass as bass
import concourse.mybir as mybir
import concourse.tile as tile
from concourse import bacc
from concourse.bass_utils import run_bass_kernel_spmd

f32 = mybir.dt.float32
bf16 = mybir.dt.bfloat16

NCORES = 8
T = 4096
C = 768
F = 3072
NH = 12
D = 64
TL = T // NCORES          # 512 local tokens per core
CCH = C // 128            # 6
FCH = F // 128            # 24
NQC = TL // 128           # 4 query chunks of 128
NSLOT = 4                 # kv slots: 128 local columns each
EPS = 1e-5
K_RANK = C * TL           # 393216 elems per rank (k^T payload)
V_RANK = TL * NH * 65     # 399360 elems per rank (V + ones columns)
SCALE = 1.0 / np.sqrt(D)


def _ap(handle, offset, pattern):
    return bass.AP(tensor=handle, offset=offset, ap=[list(p) for p in pattern])


def build_nc():
    nc = bacc.Bacc("TRN2", target_bir_lowering=False, debug=False, num_devices=NCORES)

    # ---- I/O ----
    xT_in = nc.declare_dram_parameter("xT", [C, TL], f32, isOutput=False)
    mk_in = nc.declare_dram_parameter("masks", [NCORES, 128, 128], bf16, isOutput=False)
    wq_in = nc.declare_dram_parameter("wq", [C, C], bf16, isOutput=False)
    wk_in = nc.declare_dram_parameter("wk", [C, C], bf16, isOutput=False)
    wv_in = nc.declare_dram_parameter("wv", [C, C], bf16, isOutput=False)
    wo_in = nc.declare_dram_parameter("wo", [C, C], bf16, isOutput=False)
    w1_in = nc.declare_dram_parameter("w1", [C, F], bf16, isOutput=False)
    w2_in = nc.declare_dram_parameter("w2", [F, C], bf16, isOutput=False)
    bq_in = nc.declare_dram_parameter("bq", [C], f32, isOutput=False)
    bk_in = nc.declare_dram_parameter("bk", [C], f32, isOutput=False)
    bv_in = nc.declare_dram_parameter("bv", [C], f32, isOutput=False)
    bo_in = nc.declare_dram_parameter("bo", [C], f32, isOutput=False)
    b1_in = nc.declare_dram_parameter("b1", [F], f32, isOutput=False)
    b2_in = nc.declare_dram_parameter("b2", [C], f32, isOutput=False)
    g1_in = nc.declare_dram_parameter("ln1_g", [C], f32, isOutput=False)
    h1_in = nc.declare_dram_parameter("ln1_b", [C], f32, isOutput=False)
    g2_in = nc.declare_dram_parameter("ln2_g", [C], f32, isOutput=False)
    h2_in = nc.declare_dram_parameter("ln2_b", [C], f32, isOutput=False)
    y_out = nc.declare_dram_parameter("y", [TL, C], f32, isOutput=True)

    k_loc = nc.dram_tensor("k_loc", [K_RANK], bf16)
    k_gath = nc.dram_tensor("k_gath", [NCORES * K_RANK], bf16, addr_space="Shared")
    v_loc = nc.dram_tensor("v_loc", [V_RANK], bf16)
    v_gath = nc.dram_tensor("v_gath", [NCORES * V_RANK], bf16, addr_space="Shared")

    id_f32_d = nc.inline_tensor(np.eye(128, dtype=np.float32), name="id_f32_d")
    id_bf_d = nc.inline_tensor(np.eye(128).astype(ml_dtypes.bfloat16), name="id_bf_d")

    with tile.TileContext(nc) as tc:
        import contextlib
        with contextlib.ExitStack() as ctx:
            consts = ctx.enter_context(tc.tile_pool(name="consts", bufs=1))
            xpool = ctx.enter_context(tc.tile_pool(name="xpool", bufs=1))
            actp = ctx.enter_context(tc.tile_pool(name="actp", bufs=1))
            w1pool = ctx.enter_context(tc.tile_pool(name="w1pool", bufs=1))

            # ---- load x^T ----
            xt_f = []
            for ch in range(CCH):
                tf = xpool.tile([128, TL], f32, name=f"xt_f_{ch}")
                nc.sync.dma_start(out=tf, in_=xT_in[128 * ch:128 * (ch + 1), :])
                xt_f.append(tf)

            # ---- constants ----
            id_f32 = consts.tile([128, 128], f32, name="id_f32")
            nc.sync.dma_start(out=id_f32, in_=id_f32_d[:])
            id_bf = consts.tile([128, 128], bf16, name="id_bf")
            nc.sync.dma_start(out=id_bf, in_=id_bf_d[:])
            eps_t = consts.tile([128, 1], f32, name="eps_t")
            nc.vector.memset(eps_t, EPS)
            ones64 = consts.tile([1, 64], f32, name="ones64")
            nc.vector.memset(ones64, 1.0)

            def load_bias(handle, n, name):
                t = consts.tile([128, n], f32, name=name)
                nc.sync.dma_start(out=t, in_=_ap(handle, 0, [[1, 128], [128, n]]))
                return t

            bq_sb = load_bias(bq_in, CCH, "bq_sb")
            bk_sb = load_bias(bk_in, CCH, "bk_sb")
            bo_sb = load_bias(bo_in, CCH, "bo_sb")
            b1_sb = load_bias(b1_in, FCH, "b1_sb")
            b2_sb = load_bias(b2_in, CCH, "b2_sb")
            g1_sb = load_bias(g1_in, CCH, "g1_sb")
            h1_sb = load_bias(h1_in, CCH, "h1_sb")
            g2_bc = consts.tile([128, C], f32, name="g2_bc")
            nc.sync.dma_start(out=g2_bc, in_=_ap(g2_in, 0, [[0, 128], [1, C]]))
            h2_bc = consts.tile([128, C], f32, name="h2_bc")
            nc.sync.dma_start(out=h2_bc, in_=_ap(h2_in, 0, [[0, 128], [1, C]]))
            bv_bc = consts.tile([128, C], f32, name="bv_bc")
            nc.sync.dma_start(out=bv_bc, in_=_ap(bv_in, 0, [[0, 128], [1, C]]))

            msk = []
            for r in range(NCORES):
                m = consts.tile([128, 128], bf16, name=f"msk_{r}")
                nc.sync.dma_start(out=m, in_=mk_in[r])
                msk.append(m)

            with tc.tile_pool(name="wproj", bufs=2) as wproj, \
                 tc.tile_pool(name="psumq", bufs=4, space="PSUM") as psum:
                xt_b = []
                for ch in range(CCH):
                    tb = wproj.tile([128, TL], bf16, name=f"xt_b_{ch}",
                                    tag=f"xtb_{ch}", bufs=1)
                    nc.vector.tensor_copy(out=tb, in_=xt_f[ch])
                    xt_b.append(tb)

                def load_w(handle, kch, name):
                    t = wproj.tile([128, C], bf16, name=name, tag=f"w_{kch}")
                    nc.sync.dma_start(out=t, in_=handle[128 * kch:128 * (kch + 1), :])
                    return t

                # ---- K^T first, kick its AllGather ASAP ----
                wk_t = [load_w(wk_in, kch, f"wk_{kch}") for kch in range(CCH)]
                for mch in range(CCH):
                    ps = psum.tile([128, TL], f32, name="pp", tag="pp")
                    for kch in range(CCH):
                        nc.tensor.matmul(
                            ps, lhsT=wk_t[kch][:, 128 * mch:128 * (mch + 1)],
                            rhs=xt_b[kch], start=(kch == 0), stop=(kch == CCH - 1))
                    kt = actp.tile([128, TL], bf16, name=f"kt_{mch}", tag="kt_t", bufs=2)
                    nc.vector.tensor_scalar(
                        out=kt, in0=ps, scalar1=bk_sb[:, mch:mch + 1], scalar2=None,
                        op0=mybir.AluOpType.add)
                    nc.sync.dma_start(
                        out=_ap(k_loc, mch * 128 * TL, [[TL, 128], [1, TL]]), in_=kt)
                nc.gpsimd.collective_compute(
                    "AllGather", mybir.AluOpType.bypass,
                    replica_groups=[list(range(NCORES))],
                    ins=[k_loc[:]], outs=[k_gath[:]])

                # ---- V (token-major, interleaved ones columns) + AllGather ----
                wv_t = [load_w(wv_in, kch, f"wv_{kch}") for kch in range(CCH)]
                for tch in range(NQC):
                    vt = actp.tile([128, NH, 65], bf16, name=f"v_{tch}",
                                   tag="v_t", bufs=2)
                    for nh2 in range(2):
                        ps = psum.tile([128, 384], f32, name="pv", tag="pv")
                        for kch in range(CCH):
                            nc.tensor.matmul(
                                ps,
                                lhsT=xt_b[kch][:, 128 * tch:128 * (tch + 1)],
                                rhs=wv_t[kch][:, 384 * nh2:384 * (nh2 + 1)],
                                start=(kch == 0), stop=(kch == CCH - 1))
                        nc.vector.tensor_add(
                            out=vt[:, 6 * nh2:6 * (nh2 + 1), 0:D],
                            in0=ps.rearrange("p (h d) -> p h d", d=D),
                            in1=bv_bc[:, 384 * nh2:384 * (nh2 + 1)].rearrange(
                                "p (h d) -> p h d", d=D))
                    nc.vector.memset(vt[:, :, D:D + 1], 1.0)
                    nc.sync.dma_start(
                        out=_ap(v_loc, tch * 128 * NH * 65,
                                [[NH * 65, 128], [1, NH * 65]]),
                        in_=vt)
                nc.gpsimd.collective_compute(
                    "AllGather", mybir.AluOpType.bypass,
                    replica_groups=[list(range(NCORES))],
                    ins=[v_loc[:]], outs=[v_gath[:]])

                # ---- Q^T (overlaps the collectives) ----
                wq_t = [load_w(wq_in, kch, f"wq_{kch}") for kch in range(CCH)]
                qt_b = []
                for mch in range(CCH):
                    ps = psum.tile([128, TL], f32, name="pp", tag="pp")
                    for kch in range(CCH):
                        nc.tensor.matmul(
                            ps, lhsT=wq_t[kch][:, 128 * mch:128 * (mch + 1)],
                            rhs=xt_b[kch], start=(kch == 0), stop=(kch == CCH - 1))
                    qt = actp.tile([128, TL], bf16, name=f"qt_{mch}")
                    nc.vector.tensor_scalar(
                        out=qt, in0=ps, scalar1=bq_sb[:, mch:mch + 1], scalar2=None,
                        op0=mybir.AluOpType.add)
                    qt_b.append(qt)

            # ---- prefetch W1 during attention ----
            w1_t = []
            for kch in range(CCH):
                t = w1pool.tile([128, F], bf16, name=f"w1_{kch}")
                nc.sync.dma_start(out=t, in_=w1_in[128 * kch:128 * (kch + 1), :])
                w1_t.append(t)

            aoT = [actp.tile([128, TL], bf16, name=f"aoT_{i}") for i in range(CCH)]

            with tc.tile_pool(name="kvpool", bufs=1) as kvp, \
                 tc.tile_pool(name="atw", bufs=4) as atw, \
                 tc.tile_pool(name="atp", bufs=1, space="PSUM") as atp:
                kt_g = {}
                for r in range(NCORES):
                    for ch in range(CCH):
                        t = kvp.tile([128, TL], bf16, name=f"ktg_{r}_{ch}")
                        nc.sync.dma_start(
                            out=t,
                            in_=_ap(k_gath, r * K_RANK + ch * 128 * TL,
                                    [[TL, 128], [1, TL]]))
                        kt_g[(r, ch)] = t
                va_g = {}
                for r in range(NCORES):
                    for s in range(NSLOT):
                        t = kvp.tile([128, NH, 65], bf16, name=f"vag_{r}_{s}")
                        nc.sync.dma_start(
                            out=t,
                            in_=_ap(v_gath, r * V_RANK + s * 128 * NH * 65,
                                    [[NH * 65, 128], [65, NH], [1, 65]]))
                        va_g[(r, s)] = t

                # ---- attention: head pairs at partition offsets 0/64 ----
                for hp in range(NH // 2):
                    heads = (2 * hp, 2 * hp + 1)
                    accs = {}
                    for h in heads:
                        accs[h] = atp.tile([65, TL], f32, name=f"acc{h % 2}",
                                           tag=f"acc_{h % 2}", bufs=1)
                    first = {h: True for h in heads}
                    for s in range(NSLOT):
                        q0 = 128 * s
                        nq = TL - q0
                        for r in range(NCORES):
                            lg = atp.tile([128, 2, TL], f32, name="lg",
                                          tag="lg", bufs=3)
                            for i, h in enumerate(heads):
                                ho = 64 * i
                                nc.tensor.matmul(
                                    lg[:, i, 0:nq],
                                    lhsT=kt_g[(r, hp)][ho:ho + 64,
                                                       128 * s:128 * (s + 1)],
                                    rhs=qt_b[hp][ho:ho + 64, q0:TL],
                                    start=True, stop=True)
                            pr = atw.tile([128, 2, TL], bf16, name="pr",
                                          tag="pr", bufs=3)
                            nc.scalar.activation(
                                out=pr[:, :, 0:nq], in_=lg[:, :, 0:nq],
                                func=mybir.ActivationFunctionType.Exp, scale=SCALE)
                            mb = msk[r]
                            nc.vector.tensor_mul(
                                out=pr[:, :, 0:128], in0=pr[:, :, 0:128],
                                in1=bass.AP(tensor=mb.tensor, offset=mb.offset,
                                            ap=[list(mb.ap[0]), [0, 2],
                                                list(mb.ap[1])]))
                            for i, h in enumerate(heads):
                                nc.tensor.matmul(
                                    accs[h][:, q0:TL],
                                    lhsT=va_g[(r, s)][:, h, :],
                                    rhs=pr[:, i, 0:nq],
                                    start=first[h],
                                    stop=(s == NSLOT - 1 and r == NCORES - 1))
                                first[h] = False
                    for h in heads:
                        ho = 64 * (h % 2)
                        rec = atw.tile([1, TL], f32, name="rec", tag="rec", bufs=2)
                        nc.vector.reciprocal(out=rec, in_=accs[h][D:D + 1, :])
                        brd_sb = atw.tile([64, TL], f32, name="brd_sb",
                                          tag="brd_sb", bufs=2)
                        nc.gpsimd.partition_broadcast(brd_sb, rec)
                        nc.vector.tensor_mul(
                            out=aoT[hp][ho:ho + 64, :], in0=accs[h][0:D, :],
                            in1=brd_sb)

            # ---- Wo projection + residual -> r1 (feature-major f32) ----
            postp = ctx.enter_context(tc.tile_pool(name="postp", bufs=1))
            psum2 = ctx.enter_context(tc.tile_pool(name="psum2", bufs=4, space="PSUM"))
            r1 = []
            with tc.tile_pool(name="wproj2", bufs=2) as wproj2:
                wo_t = []
                for kch in range(CCH):
                    t = wproj2.tile([128, C], bf16, name=f"wo_{kch}", tag=f"w2_{kch}")
                    nc.sync.dma_start(out=t, in_=wo_in[128 * kch:128 * (kch + 1), :])
                    wo_t.append(t)
                for mch in range(CCH):
                    ps = psum2.tile([128, TL], f32, name="pp", tag="pp")
                    for kch in range(CCH):
                        nc.tensor.matmul(
                            ps, lhsT=wo_t[kch][:, 128 * mch:128 * (mch + 1)],
                            rhs=aoT[kch], start=(kch == 0), stop=(kch == CCH - 1))
                    t = postp.tile([128, TL], f32, name=f"r1_{mch}")
                    nc.vector.tensor_scalar(
                        out=t, in0=ps, scalar1=bo_sb[:, mch:mch + 1], scalar2=None,
                        op0=mybir.AluOpType.add)
                    nc.vector.tensor_add(out=t, in0=t, in1=xt_f[mch])
                    r1.append(t)

            # ---- LN1 (token-major via transposes), output h^T bf16 ----
            hT = [postp.tile([128, TL], bf16, name=f"hT_{i}") for i in range(CCH)]
            with tc.tile_pool(name="lnw", bufs=3) as lnw, \
                 tc.tile_pool(name="lnp", bufs=2, space="PSUM") as lnp:
                for tch in range(NQC):
                    tm = lnw.tile([128, C], f32, name="tm", tag="tm")
                    for ch in range(CCH):
                        tp = lnp.tile([128, 128], f32, name="tp", tag="tp", bufs=2)
                        nc.tensor.transpose(
                            tp, in_=r1[ch][:, 128 * tch:128 * (tch + 1)],
                            identity=id_f32)
                        nc.vector.tensor_copy(
                            out=tm[:, 128 * ch:128 * (ch + 1)], in_=tp)
                    st = lnw.tile([128, 3, 6], f32, name="st", tag="st")
                    for sg in range(3):
                        nc.vector.bn_stats(
                            out=st[:, sg, :], in_=tm[:, 256 * sg:256 * (sg + 1)])
                    mv = lnw.tile([128, 2], f32, name="mv", tag="mv")
                    nc.vector.bn_aggr(out=mv, in_=st)
                    sd = lnw.tile([128, 1], f32, name="sd", tag="sd")
                    nc.scalar.activation(
                        out=sd, in_=mv[:, 1:2],
                        func=mybir.ActivationFunctionType.Sqrt, bias=eps_t, scale=1.0)
                    rs = lnw.tile([128, 1], f32, name="rs", tag="rs")
                    nc.vector.reciprocal(out=rs, in_=sd)
                    tn = lnw.tile([128, C], bf16, name="tn", tag="tn")
                    nc.vector.tensor_scalar(
                        out=tn, in0=tm, scalar1=mv[:, 0:1], scalar2=rs,
                        op0=mybir.AluOpType.subtract, op1=mybir.AluOpType.mult)
                    for ch in range(CCH):
                        tp2 = lnp.tile([128, 128], bf16, name="tp2", tag="tp2", bufs=2)
                        nc.tensor.transpose(
                            tp2, in_=tn[:, 128 * ch:128 * (ch + 1)], identity=id_bf)
                        nc.vector.tensor_scalar(
                            out=hT[ch][:, 128 * tch:128 * (tch + 1)], in0=tp2,
                            scalar1=g1_sb[:, ch:ch + 1], scalar2=h1_sb[:, ch:ch + 1],
                            op0=mybir.AluOpType.mult, op1=mybir.AluOpType.add)

            # ---- MLP ----
            with tc.tile_pool(name="mlpw", bufs=1) as mlpw:
                mup = [mlpw.tile([128, TL], bf16, name=f"mup_{i}") for i in range(FCH)]
                w2_t = []
                for kch in range(FCH):
                    t = mlpw.tile([128, C], bf16, name=f"w2t_{kch}")
                    nc.sync.dma_start(out=t, in_=w2_in[128 * kch:128 * (kch + 1), :])
                    w2_t.append(t)
                for mch in range(FCH):
                    ps = psum2.tile([128, TL], f32, name="pp", tag="pp")
                    for kch in range(CCH):
                        nc.tensor.matmul(
                            ps, lhsT=w1_t[kch][:, 128 * mch:128 * (mch + 1)],
                            rhs=hT[kch], start=(kch == 0), stop=(kch == CCH - 1))
                    nc.scalar.activation(
                        out=mup[mch], in_=ps,
                        func=mybir.ActivationFunctionType.Gelu,
                        bias=b1_sb[:, mch:mch + 1], scale=1.0)

                r2 = []
                for mch in range(CCH):
                    ps = psum2.tile([128, TL], f32, name="pp", tag="pp")
                    for kch in range(FCH):
                        nc.tensor.matmul(
                            ps, lhsT=w2_t[kch][:, 128 * mch:128 * (mch + 1)],
                            rhs=mup[kch], start=(kch == 0), stop=(kch == FCH - 1))
                    t = postp.tile([128, TL], f32, name=f"r2_{mch}")
                    nc.vector.tensor_scalar(
                        out=t, in0=ps, scalar1=b2_sb[:, mch:mch + 1], scalar2=None,
                        op0=mybir.AluOpType.add)
                    nc.vector.tensor_add(out=t, in0=t, in1=hT[mch])
                    r2.append(t)

                # ---- LN2 (token-major, gain/bias applied token-major) -> y ----
                with tc.tile_pool(name="lnw2", bufs=3) as lnw2, \
                     tc.tile_pool(name="lnp2", bufs=2, space="PSUM") as lnp2:
                    for tch in range(NQC):
                        tm = lnw2.tile([128, C], f32, name="tm2", tag="tm2")
                        for ch in range(CCH):
                            tp = lnp2.tile([128, 128], f32, name="tp3",
                                           tag="tp3", bufs=2)
                            nc.tensor.transpose(
                                tp, in_=r2[ch][:, 128 * tch:128 * (tch + 1)],
                                identity=id_f32)
                            nc.vector.tensor_copy(
                                out=tm[:, 128 * ch:128 * (ch + 1)], in_=tp)
                        st = lnw2.tile([128, 3, 6], f32, name="st2", tag="st2")
                        for sg in range(3):
                            nc.vector.bn_stats(
                                out=st[:, sg, :], in_=tm[:, 256 * sg:256 * (sg + 1)])
                        mv = lnw2.tile([128, 2], f32, name="mv2", tag="mv2")
                        nc.vector.bn_aggr(out=mv, in_=st)
                        sd = lnw2.tile([128, 1], f32, name="sd2", tag="sd2")
                        nc.scalar.activation(
                            out=sd, in_=mv[:, 1:2],
                            func=mybir.ActivationFunctionType.Sqrt,
                            bias=eps_t, scale=1.0)
                        rs = lnw2.tile([128, 1], f32, name="rs2", tag="rs2")
                        nc.vector.reciprocal(out=rs, in_=sd)
                        tn = lnw2.tile([128, C], f32, name="tn2", tag="tn2")
                        nc.vector.tensor_scalar(
                            out=tn, in0=tm, scalar1=mv[:, 0:1], scalar2=rs,
                            op0=mybir.AluOpType.subtract, op1=mybir.AluOpType.mult)
                        yt = lnw2.tile([128, C], f32, name="yt", tag="yt")
                        nc.vector.tensor_mul(out=yt, in0=tn, in1=g2_bc)
                        nc.vector.tensor_add(out=yt, in0=yt, in1=h2_bc)
                        nc.sync.dma_start(
                            out=y_out[128 * tch:128 * (tch + 1), :], in_=yt)

    nc.compile()
    return nc


_NC_CACHE = None


def _get_nc():
    global _NC_CACHE
    if _NC_CACHE is None:
        _NC_CACHE = build_nc()
    return _NC_CACHE


def make_in_maps(inputs):
    x = np.asarray(inputs["x"], dtype=np.float32)      # [1, T, C]
    to_bf = lambda a: np.asarray(a, dtype=np.float32).astype(ml_dtypes.bfloat16)
    shared = {
        "wq": to_bf(inputs["Wq"]), "wk": to_bf(inputs["Wk"]),
        "wv": to_bf(inputs["Wv"]), "wo": to_bf(inputs["Wo"]),
        "w1": to_bf(inputs["W1"]), "w2": to_bf(inputs["W2"]),
        "bq": np.asarray(inputs["bq"], np.float32),
        "bk": np.asarray(inputs["bk"], np.float32),
        "bv": np.asarray(inputs["bv"], np.float32),
        "bo": np.asarray(inputs["bo"], np.float32),
        "b1": np.asarray(inputs["b1"], np.float32),
        "b2": np.asarray(inputs["b2"], np.float32),
        "ln1_g": np.asarray(inputs["ln1_g"], np.float32),
        "ln1_b": np.asarray(inputs["ln1_b"], np.float32),
        "ln2_g": np.asarray(inputs["ln2_g"], np.float32),
        "ln2_b": np.asarray(inputs["ln2_b"], np.float32),
    }
    ki = np.arange(128)[:, None]
    qi = np.arange(128)[None, :]
    in_maps = []
    for c in range(NCORES):
        xT = np.ascontiguousarray(x[0, c::NCORES, :].T)        # [C, TL]
        # multiplicative 0/1 causal masks for the diagonal kv slot
        masks = np.stack([
            (8 * ki + r <= 8 * qi + c) for r in range(NCORES)
        ]).astype(ml_dtypes.bfloat16)                           # [8, 128, 128]
        m = dict(shared)
        m["xT"] = xT
        m["masks"] = masks
        in_maps.append(m)
    return in_maps


def kernel(**inputs):
    nc = _get_nc()
    in_maps = make_in_maps(inputs)
    res = run_bass_kernel_spmd(nc, in_maps, list(range(NCORES)))
    x = np.asarray(inputs["x"])
    out = np.empty((1, T, C), dtype=np.float32)
    for c in range(NCORES):
        out[0, c::NCORES, :] = res.results[c]["y"]
    return out.astype(x.dtype) if x.dtype != np.float32 else out
